# revision 61
# baseline (speedup 1.0000x reference)
"""Multi-head attention (B=2, S=2048, D=2048, H=16) on 8 Trainium2 cores.

Sharding: core c handles batch b=c//4 and head-group g=c%4 (4 heads, 512
features). Everything on-device runs in a transposed layout so the PE
contraction dim is always the partition dim:

  qT,kT [C=512, S]   = W[g].T-slices applied to hsT
  v     [S, C=512]
  scoresT[sk, sq]    = kT_h.T @ qT_h        (per head, per sq-chunk)
  p = exp(scoresT/sqrt(128))                (no max-subtract: scores are O(5))
  outT_h[c, sq]      = v_h.T @ p            (accumulated over sk blocks)
  l_h[sq] = sum_sk p -> r=1/l, attnT_h *= r
  partialT[n, sq]    = wo_g.T @ attnT       (per-core partial of o-proj)

Host sums the 4 per-batch partials (fp16 on wire), transposes back, adds bo.
All matmuls run in fp16 (4x fp32 rate).

Phases A (q/k-proj) and B (v-proj) are merged: per 512-column chunk of
hsT we run a q-pass, k-pass and v-pass against the same SBUF-resident hs
tile (PSUM banks rotate between passes), so hs is loaded once. Inputs are
DMA'd in a handful of large descriptors on the sync queue (host pre-packs
the tile layout); outputs go out in fp16 per 128-row block on gpsimd.
"""
import sys

if "/opt/trn_rl_repo" not in sys.path:
    sys.path.insert(0, "/opt/trn_rl_repo")

import numpy as np

B, S, D, H = 2, 2048, 2048, 16
HD = 128          # head dim
C = 512           # features per core (4 heads)
NB = S // 128     # 16 partition blocks
CH = S // 512     # 4 free-dim chunks
SCALE = 1.0 / np.sqrt(HD)

_BUILT = {}


def _build_program():
    import concourse.bass as bass
    import concourse.tile as tile
    from concourse import bacc, mybir, bass_isa
    from contextlib import ExitStack

    f32 = mybir.dt.float32
    f16 = mybir.dt.float16

    nc = bacc.Bacc("TRN2", target_bir_lowering=False, debug=False, num_devices=1)
    # Host pre-packs every input into the exact SBUF tile layout so each
    # DMA below is a plain 2D contiguous transfer.
    #   hs4[p, ch*8192 + d*512 + c] = hsT[d*128+p, ch*512+c]
    #   wq3/wk3/wv3[p, d*512 + c]   = W.T[d*128+p, c]          (per-core 512 cols)
    #   wo4[p, cb*2048 + n]         = Wo_g.T[cb*128+p, n]
    hs4 = nc.dram_tensor("hs4", (128, CH * NB * 512), f16, kind="ExternalInput").ap()
    wq3 = nc.dram_tensor("wq3", (128, NB * 512), f16, kind="ExternalInput").ap()
    wk3 = nc.dram_tensor("wk3", (128, NB * 512), f16, kind="ExternalInput").ap()
    wv3 = nc.dram_tensor("wv3", (128, NB * 512), f16, kind="ExternalInput").ap()
    wo4 = nc.dram_tensor("wo4", (128, 4 * S), f16, kind="ExternalInput").ap()
    outT = nc.dram_tensor("outT", (D, S), f16, kind="ExternalOutput").ap()

    with tile.TileContext(nc) as tc, ExitStack() as top:
        dma_in = nc.sync.dma_start
        dma_out = nc.gpsimd.dma_start

        # ---- persistent activations -------------------------------------
        # attnT[h] reuses qT[h]'s slot (same tag): qT[h] is dead after head
        # h's last scores matmul, right when attnT[h] starts filling.
        qk_pool = top.enter_context(tc.tile_pool(name="qk", bufs=1))
        qT = [qk_pool.tile([128, S], f16, tag=f"qT{cb}", name=f"qT{cb}") for cb in range(4)]
        kT = [qk_pool.tile([128, S], f16, tag=f"kT{cb}", name=f"kT{cb}") for cb in range(4)]
        v_pool = top.enter_context(tc.tile_pool(name="v", bufs=1))
        v_sb = [v_pool.tile([128, C], f16, tag=f"v{sb}", name=f"v{sb}") for sb in range(NB)]
        wop = top.enter_context(tc.tile_pool(name="wo", bufs=1))
        wo_sb = wop.tile([128, 4 * S], f16, name="wo_sb")
        attnT = []

        # ---- phase AB: qT, kT, v ----------------------------------------
        # wv and the last hs chunk outlive phase AB: the ch3 v-pass is
        # interleaved into head 0's first half (phase C), where the scalar
        # exp otherwise leaves the PE ~25% idle.
        wvp = top.enter_context(tc.tile_pool(name="wv2", bufs=1))
        wv_sb = wvp.tile([128, NB * 512], f16, name="wv_sb")
        h3p = top.enter_context(tc.tile_pool(name="hs3p", bufs=1))
        with tc.tile_pool(name="w", bufs=1) as wpool, \
             tc.tile_pool(name="hs", bufs=2) as hpool, \
             tc.tile_pool(name="psAB", bufs=1, space="PSUM") as psAB:
            wq_sb = wpool.tile([128, NB * 512], f16, name="wq_sb")
            wk_sb = wpool.tile([128, NB * 512], f16, name="wk_sb")
            # staged input DMAs: wq and hs chunk 0 stream in matched,
            # interleaved d-range pieces so pass-0 matmul progress tracks
            # transfer progress instead of waiting on a monolithic load.
            hs_t0 = hpool.tile([128, NB * 512], f16, tag="hs")
            for lo, hi in ((0, 1), (1, 3), (3, 6), (6, 11), (11, 16)):
                dma_in(wq_sb[:, lo * 512:hi * 512], wq3[:, lo * 512:hi * 512])
                dma_in(hs_t0[:, lo * 512:hi * 512], hs4[:, lo * 512:hi * 512])
            dma_in(wk_sb[:], wk3[:])
            dma_in(wv_sb[:], wv3[:])
            dma_in(wo_sb[:], wo4[:])

            pass_idx = 0
            hs_t = hs_t0
            for ch in range(CH):
                if ch + 1 < CH:
                    if ch + 1 == CH - 1:
                        hs_nxt = h3p.tile([128, NB * 512], f16, name="hs3")
                        hs3_t = hs_nxt
                    else:
                        hs_nxt = hpool.tile([128, NB * 512], f16, tag="hs")
                    dma_in(hs_nxt[:], hs4[:, (ch + 1) * 8192:(ch + 2) * 8192])
                # q-pass, k-pass: out [128 c-feat, 512 sq], contract over d.
                # ch3 runs k first so the last pass's psum banks are freed
                # by q's vector copies rather than k's scalar copies.
                order = (("q", wq_sb, qT), ("k", wk_sb, kT))
                if ch == CH - 1:
                    order = (order[1], order[0])
                for which, w_t, dstT in order:
                    base = 4 * (pass_idx % 2)
                    ps = [psAB.tile([128, 512], f32, tag=f"T{base+i}", name=f"p{which}{i}")
                          for i in range(4)]
                    for d in range(NB):
                        for cb in range(4):
                            nc.tensor.matmul(
                                ps[cb][:],
                                lhsT=w_t[:, d * 512 + cb * 128:d * 512 + (cb + 1) * 128],
                                rhs=hs_t[:, d * 512:(d + 1) * 512],
                                start=(d == 0), stop=(d == NB - 1))
                    for cb in range(4):
                        dst = dstT[cb][:, ch * 512:(ch + 1) * 512]
                        if ch == CH - 1 and which == "q":
                            # final pass: split copies across engines so the
                            # AB->C psum-pool handoff barrier (which waits on
                            # the last copy) clears ~1.4us sooner.
                            if cb < 2:
                                nc.vector.tensor_copy(dst, ps[cb][:])
                            else:
                                nc.scalar.copy(dst, ps[cb][:])
                        elif which == "q":
                            nc.vector.tensor_copy(dst, ps[cb][:])
                        else:
                            nc.scalar.copy(dst, ps[cb][:])
                    pass_idx += 1
                # v-pass: out [128 s-pos, 512 c-feat], contract over d.
                # ch3's v-pass is deferred into phase C (head 0, half 0).
                if ch < CH - 1:
                    base = 4 * (pass_idx % 2)
                    pv = [psAB.tile([128, 512], f32, tag=f"T{base+j}", name=f"pv{j}")
                          for j in range(4)]
                    for d in range(NB):
                        for j in range(4):
                            nc.tensor.matmul(
                                pv[j][:],
                                lhsT=hs_t[:, d * 512 + j * 128:d * 512 + (j + 1) * 128],
                                rhs=wv_sb[:, d * 512:(d + 1) * 512],
                                start=(d == 0), stop=(d == NB - 1))
                    for j in range(4):
                        nc.vector.tensor_copy(v_sb[ch * 4 + j][:], pv[j][:])
                    pass_idx += 1
                hs_t = hs_nxt if ch + 1 < CH else None

        # ---- phase C: attention (+ interleaved D-partial for heads 0,1) --
        # Each head runs in two sq-halves of 1024 so the av accumulator is
        # a single 2-bank PSUM slot; the 2 banks this frees host a small
        # o-proj pipeline: during heads 2 and 3 (whose exp keeps the scalar
        # engine saturated and the PE ~25% idle), one (nb,ch) cell of the
        # wo-contraction over heads {0,1} runs per sk step and is flushed
        # fp16 to SBUF (o1). Phase D then only contracts heads {2,3} and
        # merges o1 during the psum read-out.
        o1_pool = top.enter_context(tc.tile_pool(name="o1", bufs=1))
        o1 = [o1_pool.tile([128, S], f16, tag=f"o1_{nb}", name=f"o1_{nb}")
              for nb in range(NB)]
        # aT gets its own tiles: with sq-halving, half 0's normalize writes
        # aT while qT[h] is still read by half 1's scores, so the v1 trick
        # of reusing qT[h]'s slot would deadlock the pool.
        aT_pool = top.enter_context(tc.tile_pool(name="aT", bufs=1))
        HF = S // 2
        with tc.tile_pool(name="esb", bufs=6) as epool, \
             tc.tile_pool(name="lwork", bufs=2) as lpool, \
             tc.tile_pool(name="ones", bufs=1) as onepool, \
             tc.tile_pool(name="psS", bufs=2, space="PSUM") as psS, \
             tc.tile_pool(name="psO", bufs=1, space="PSUM") as psO, \
             tc.tile_pool(name="psD1", bufs=2, space="PSUM") as psD1:
            ones_f = onepool.tile([128, 128], f32, name="ones_f")
            nc.vector.memset(ones_f[:], 1.0)
            ones = onepool.tile([128, 128], f16, name="ones")
            nc.vector.tensor_copy(ones[:], ones_f[:])
            for h in range(4):
                aT = aT_pool.tile([128, S], f16, tag=f"aT{h}", name=f"aT{h}")
                deferred_flush = []
                for half in range(2):
                    qo = half * HF
                    # head 0, half 0 carries the deferred ch3 v-pass: 4
                    # v-matmuls per sk step fill the PE while exp runs; the
                    # av matmuls for sk 12-15 (whose v rows are produced
                    # here) are deferred past v completion so an in-order
                    # PE stall cannot deadlock against v production.
                    vfuse = (h == 0 and half == 0)
                    e_defer = {}
                    po = psO.tile([128, HF], f32, tag="po", name=f"po{h}_{half}")
                    acc = lpool.tile([128, HF], f16, tag="lw", name="acc")
                    for sk in range(NB):
                        ks = kT[h][:, sk * 128:(sk + 1) * 128]
                        ps = psS.tile([128, HF], f32, tag="ps", name="ps")
                        for c2 in range(2):
                            nc.tensor.matmul(
                                ps[:, c2 * 512:(c2 + 1) * 512],
                                lhsT=ks, rhs=qT[h][:, qo + c2 * 512:qo + (c2 + 1) * 512],
                                start=True, stop=True)
                        e_t = epool.tile([128, HF], f16, tag="esb")
                        nc.scalar.activation(e_t[:], ps[:],
                                             mybir.ActivationFunctionType.Exp,
                                             scale=float(SCALE))
                        vs = v_sb[sk][:, h * 128:(h + 1) * 128]
                        if vfuse and sk >= 12:
                            e_defer[sk] = e_t
                        else:
                            for c2 in range(2):
                                nc.tensor.matmul(
                                    po[:, c2 * 512:(c2 + 1) * 512], lhsT=vs,
                                    rhs=e_t[:, c2 * 512:(c2 + 1) * 512],
                                    start=(sk == 0),
                                    stop=(not vfuse and sk == NB - 1))
                        if vfuse:
                            for t in range(4):
                                vi = sk * 4 + t
                                j, dd = divmod(vi, NB)
                                if dd == 0:
                                    pv_c = psD1.tile([128, 512], f32, tag="pp",
                                                     name=f"pvc{j}")
                                nc.tensor.matmul(
                                    pv_c[:],
                                    lhsT=hs3_t[:, dd * 512 + j * 128:dd * 512 + (j + 1) * 128],
                                    rhs=wv_sb[:, dd * 512:(dd + 1) * 512],
                                    start=(dd == 0), stop=(dd == NB - 1))
                                if dd == NB - 1:
                                    nc.vector.tensor_copy(v_sb[12 + j][:], pv_c[:])
                        if h >= 2:
                            # one o-proj cell (heads 0,1) per step in the
                            # spare psum banks; flushes fp16 to o1, mostly
                            # on vector but every 4th on scalar (which has
                            # ~180ns/step of slack under the PE-bound rate).
                            ci = (h - 2) * 32 + half * 16 + sk
                            nb_i, ch_i = divmod(ci, 4)
                            pp = psD1.tile([128, 512], f32, tag="pp", name="pp")
                            for cb in (0, 1):
                                nc.tensor.matmul(
                                    pp[:],
                                    lhsT=wo_sb[:, cb * 2048 + nb_i * 128:cb * 2048 + (nb_i + 1) * 128],
                                    rhs=attnT[cb][:, ch_i * 512:(ch_i + 1) * 512],
                                    start=(cb == 0), stop=(cb == 1))
                            dst = o1[nb_i][:, ch_i * 512:(ch_i + 1) * 512]
                            if h == 3 and half == 1 and sk >= NB - 2:
                                # keep the last flushes out of vector's
                                # queue ahead of the reciprocal chain that
                                # gates phase D's psum banks.
                                deferred_flush.append((dst, pp))
                            else:
                                nc.vector.tensor_copy(dst, pp[:])
                        # acc skips sk15: its contribution enters the l-sum
                        # directly via a second accumulating ones-matmul, so
                        # the reciprocal chain starts one period earlier.
                        if sk == 0:
                            nc.vector.tensor_copy(acc[:], e_t[:])
                        elif sk < NB - 1:
                            nc.vector.tensor_add(acc[:], acc[:], e_t[:])
                        else:
                            e_last = e_t
                    # l-sums: for heads 0,1 they live in the (otherwise
                    # idle) psD1 banks so the scores ping-pong never waits
                    # on the reciprocal chain; for heads 2,3 psD1 hosts the
                    # o-proj cells (which must not wait on recips — a
                    # stalled cell matmul blocks the in-order PE), so the
                    # l-sums use a scores slot there instead.
                    # aT = po * (1/l) is fused into the psum read-out.
                    if h < 2:
                        pl = [psD1.tile([128, 512], f32, tag="pp", name=f"pl{i}")
                              for i in range(2)]
                        pls = [pl[0][:], pl[1][:]]
                    else:
                        plw = psS.tile([128, HF], f32, tag="ps", name="pl")
                        pls = [plw[:, 0:512], plw[:, 512:1024]]
                    for c2 in range(2):
                        nc.tensor.matmul(pls[c2], lhsT=ones[:],
                                         rhs=acc[:, c2 * 512:(c2 + 1) * 512],
                                         start=True, stop=False)
                        nc.tensor.matmul(pls[c2], lhsT=ones[:],
                                         rhs=e_last[:, c2 * 512:(c2 + 1) * 512],
                                         start=False, stop=True)
                    # catch-up av for the v rows produced in this half
                    for sk in sorted(e_defer):
                        vs = v_sb[sk][:, h * 128:(h + 1) * 128]
                        for c2 in range(2):
                            nc.tensor.matmul(
                                po[:, c2 * 512:(c2 + 1) * 512], lhsT=vs,
                                rhs=e_defer[sk][:, c2 * 512:(c2 + 1) * 512],
                                start=False, stop=(sk == NB - 1))
                    rcp = lpool.tile([128, HF], f32, tag="lwf", name="rcp")
                    if h < 2:
                        for c2 in range(2):
                            nc.vector.reciprocal_approx_fast(
                                rcp[:, c2 * 512:(c2 + 1) * 512], pls[c2])
                    else:
                        # one wide recip: both l halves sit in one psS tile,
                        # and this chain (recips -> aTmul) is the serial
                        # vector tail at every h2/h3 half boundary.
                        nc.vector.reciprocal_approx_fast(rcp[:], plw[:])
                    nc.vector.tensor_mul(aT[:, qo:qo + HF], po[:], rcp[:])
                    # deferred flushes go to SCALAR: at h3's end there are no
                    # more exps, so scalar is idle and these run in parallel
                    # with vector's recip/normalize chain — the last psum
                    # reader that gates phase D's pool handoff.
                    for dst, pp_d in deferred_flush:
                        nc.scalar.copy(dst, pp_d[:])
                    deferred_flush = []
                attnT.append(aT)

        # ---- phase D: o-proj for heads {2,3} + merge with o1 (fp16 out) --
        with tc.tile_pool(name="osb", bufs=3) as opool, \
             tc.tile_pool(name="tmp", bufs=2) as tpool, \
             tc.tile_pool(name="psD", bufs=2, space="PSUM") as psD:
            for nb in range(NB):
                pp = [psD.tile([128, 1024], f32, tag=f"pp{i}", name=f"pp{i}") for i in range(2)]
                for cb in (2, 3):
                    for ch in range(CH):
                        nc.tensor.matmul(
                            pp[ch // 2][:, (ch % 2) * 512:(ch % 2) * 512 + 512],
                            lhsT=wo_sb[:, cb * 2048 + nb * 128:cb * 2048 + (nb + 1) * 128],
                            rhs=attnT[cb][:, ch * 512:(ch + 1) * 512],
                            start=(cb == 2), stop=(cb == 3))
                o_t = opool.tile([128, S], f16, tag="osb")
                # merge heads{2,3} psum with o1 (heads{0,1}) -> fp16 out.
                # Balance the slow 1-elem/cycle psum reads across scalar and
                # vector: every 3rd block goes scalar-copy x2 + one wide
                # fp16 add; the rest split one psum-TT on vector and one
                # scalar copy + fp16 add. Both engines land ~25us < PE 27.7.
                if nb % 3 == 0:
                    tmp2 = tpool.tile([128, S], f16, tag="tmp2")
                    nc.scalar.copy(tmp2[:, 0:1024], pp[0][:])
                    nc.scalar.copy(tmp2[:, 1024:2048], pp[1][:])
                    nc.vector.tensor_add(o_t[:], tmp2[:], o1[nb][:])
                else:
                    nc.vector.tensor_add(o_t[:, 0:1024], pp[0][:], o1[nb][:, 0:1024])
                    tmp = tpool.tile([128, 1024], f16, tag="tmp")
                    nc.scalar.copy(tmp[:], pp[1][:])
                    nc.vector.tensor_add(o_t[:, 1024:2048], tmp[:], o1[nb][:, 1024:2048])
                if nb >= NB - 2:
                    # last blocks: per-half DMAs so the tail isn't one
                    # full-row transfer behind the final merges.
                    for i in range(2):
                        dma_out(outT[nb * 128:(nb + 1) * 128, i * 1024:(i + 1) * 1024],
                                o_t[:, i * 1024:(i + 1) * 1024])
                else:
                    dma_out(outT[nb * 128:(nb + 1) * 128, :], o_t[:])

    nc.compile()
    return nc


def _get_program():
    if "nc" not in _BUILT:
        _BUILT["nc"] = _build_program()
    return _BUILT["nc"]


def _make_in_maps(hs, Wq, Wk, Wv, Wo):
    """Host-side pack of full fp32 inputs into 8 per-core fp16 tile layouts."""
    in_maps = []
    hs16 = [None] * B
    for b in range(B):
        hs16[b] = np.ascontiguousarray(
            hs[b].T.reshape(NB, 128, CH, 512).transpose(1, 2, 0, 3).reshape(128, CH * NB * 512)
        ).astype(np.float16)
    for c in range(8):
        b, g = divmod(c, 4)
        sl = slice(g * C, (g + 1) * C)
        wq = Wq[sl, :].T.reshape(NB, 128, C).transpose(1, 0, 2).reshape(128, NB * C)
        wk = Wk[sl, :].T.reshape(NB, 128, C).transpose(1, 0, 2).reshape(128, NB * C)
        wv = Wv[sl, :].T.reshape(NB, 128, C).transpose(1, 0, 2).reshape(128, NB * C)
        wo = Wo[:, sl].T.reshape(4, 128, S).transpose(1, 0, 2).reshape(128, 4 * S)
        in_maps.append({
            "hs4": hs16[b],
            "wq3": np.ascontiguousarray(wq).astype(np.float16),
            "wk3": np.ascontiguousarray(wk).astype(np.float16),
            "wv3": np.ascontiguousarray(wv).astype(np.float16),
            "wo4": np.ascontiguousarray(wo).astype(np.float16),
        })
    return in_maps


def _reference_fallback(hidden_states, attention_mask, Wq, bq, Wk, bk, Wv, bv, Wo, bo):
    q = hidden_states @ Wq.T + bq
    k = hidden_states @ Wk.T + bk
    v = hidden_states @ Wv.T + bv
    q = q.reshape(B, S, H, HD).transpose(0, 2, 1, 3)
    k = k.reshape(B, S, H, HD).transpose(0, 2, 1, 3)
    v = v.reshape(B, S, H, HD).transpose(0, 2, 1, 3)
    scores = np.einsum("bhqd,bhkd->bhqk", q, k) / np.sqrt(np.float32(HD))
    scores = scores + attention_mask
    scores -= scores.max(axis=-1, keepdims=True)
    e = np.exp(scores)
    attn = e / e.sum(axis=-1, keepdims=True)
    out = np.einsum("bhqk,bhkd->bhqd", attn, v)
    out = out.transpose(0, 2, 1, 3).reshape(B, S, D)
    return (out @ Wo.T + bo).astype(np.float32)


def kernel(hidden_states, attention_mask, Wq, bq, Wk, bk, Wv, bv, Wo, bo):
    from concourse import bass_utils

    hs = np.ascontiguousarray(np.asarray(hidden_states, dtype=np.float32))
    mask = np.asarray(attention_mask, dtype=np.float32)
    Wq = np.asarray(Wq, dtype=np.float32)
    Wk = np.asarray(Wk, dtype=np.float32)
    Wv = np.asarray(Wv, dtype=np.float32)
    Wo = np.asarray(Wo, dtype=np.float32)
    bq = np.asarray(bq, dtype=np.float32)
    bk = np.asarray(bk, dtype=np.float32)
    bv = np.asarray(bv, dtype=np.float32)
    bo = np.asarray(bo, dtype=np.float32)

    # Device program hardcodes zero mask / zero qkv biases (true for this
    # problem's setup_inputs); fall back to exact math if that ever changes.
    if mask.any() or bq.any() or bk.any() or bv.any():
        return _reference_fallback(hs, mask, Wq, bq, Wk, bk, Wv, bv, Wo, bo)

    nc = _get_program()
    in_maps = _make_in_maps(hs, Wq, Wk, Wv, Wo)
    res = bass_utils.run_bass_kernel_spmd(nc, in_maps, core_ids=list(range(8)))

    out = np.empty((B, S, D), dtype=np.float32)
    for b in range(B):
        accT = res.results[b * 4 + 0]["outT"].astype(np.float32)
        for g in range(1, 4):
            accT = accT + res.results[b * 4 + g]["outT"].astype(np.float32)
        out[b] = accT.T + bo
    return out


# revision 62
# speedup vs baseline: 1.0062x; 1.0062x over previous
"""Multi-head attention (B=2, S=2048, D=2048, H=16) on 8 Trainium2 cores.

Sharding: core c handles batch b=c//4 and head-group g=c%4 (4 heads, 512
features). Everything on-device runs in a transposed layout so the PE
contraction dim is always the partition dim:

  qT,kT [C=512, S]   = W[g].T-slices applied to hsT
  v     [S, C=512]
  scoresT[sk, sq]    = kT_h.T @ qT_h        (per head, per sq-chunk)
  p = exp(scoresT/sqrt(128))                (no max-subtract: scores are O(5))
  outT_h[c, sq]      = v_h.T @ p            (accumulated over sk blocks)
  l_h[sq] = sum_sk p -> r=1/l, attnT_h *= r
  partialT[n, sq]    = wo_g.T @ attnT       (per-core partial of o-proj)

Host sums the 4 per-batch partials (fp16 on wire), transposes back, adds bo.
All matmuls run in fp16 (4x fp32 rate).

Phases A (q/k-proj) and B (v-proj) are merged: per 512-column chunk of
hsT we run a q-pass, k-pass and v-pass against the same SBUF-resident hs
tile (PSUM banks rotate between passes), so hs is loaded once. Inputs are
DMA'd in a handful of large descriptors on the sync queue (host pre-packs
the tile layout); outputs go out in fp16 per 128-row block on gpsimd.
"""
import sys

if "/opt/trn_rl_repo" not in sys.path:
    sys.path.insert(0, "/opt/trn_rl_repo")

import numpy as np

B, S, D, H = 2, 2048, 2048, 16
HD = 128          # head dim
C = 512           # features per core (4 heads)
NB = S // 128     # 16 partition blocks
CH = S // 512     # 4 free-dim chunks
SCALE = 1.0 / np.sqrt(HD)

_BUILT = {}


def _build_program():
    import concourse.bass as bass
    import concourse.tile as tile
    from concourse import bacc, mybir, bass_isa
    from contextlib import ExitStack

    f32 = mybir.dt.float32
    f16 = mybir.dt.float16

    nc = bacc.Bacc("TRN2", target_bir_lowering=False, debug=False, num_devices=1)
    # Host pre-packs every input into the exact SBUF tile layout so each
    # DMA below is a plain 2D contiguous transfer.
    #   hs4[p, ch*8192 + d*512 + c] = hsT[d*128+p, ch*512+c]
    #   wq3/wk3/wv3[p, d*512 + c]   = W.T[d*128+p, c]          (per-core 512 cols)
    #   wo4[p, cb*2048 + n]         = Wo_g.T[cb*128+p, n]
    hs4 = nc.dram_tensor("hs4", (128, CH * NB * 512), f16, kind="ExternalInput").ap()
    wq3 = nc.dram_tensor("wq3", (128, NB * 512), f16, kind="ExternalInput").ap()
    wk3 = nc.dram_tensor("wk3", (128, NB * 512), f16, kind="ExternalInput").ap()
    wv3 = nc.dram_tensor("wv3", (128, NB * 512), f16, kind="ExternalInput").ap()
    wo4 = nc.dram_tensor("wo4", (128, 4 * S), f16, kind="ExternalInput").ap()
    outT = nc.dram_tensor("outT", (D, S), f16, kind="ExternalOutput").ap()

    with tile.TileContext(nc) as tc, ExitStack() as top:
        dma_in = nc.sync.dma_start
        dma_out = nc.gpsimd.dma_start

        # ---- persistent activations -------------------------------------
        # attnT[h] reuses qT[h]'s slot (same tag): qT[h] is dead after head
        # h's last scores matmul, right when attnT[h] starts filling.
        qk_pool = top.enter_context(tc.tile_pool(name="qk", bufs=1))
        qT = [qk_pool.tile([128, S], f16, tag=f"qT{cb}", name=f"qT{cb}") for cb in range(4)]
        kT = [qk_pool.tile([128, S], f16, tag=f"kT{cb}", name=f"kT{cb}") for cb in range(4)]
        v_pool = top.enter_context(tc.tile_pool(name="v", bufs=1))
        v_sb = [v_pool.tile([128, C], f16, tag=f"v{sb}", name=f"v{sb}") for sb in range(NB)]
        wop = top.enter_context(tc.tile_pool(name="wo", bufs=1))
        wo_sb = wop.tile([128, 4 * S], f16, name="wo_sb")
        attnT = []

        # ---- phase AB: qT, kT, v ----------------------------------------
        # wv and the last hs chunk outlive phase AB: the ch3 v-pass is
        # interleaved into head 0's first half (phase C), where the scalar
        # exp otherwise leaves the PE ~25% idle.
        wvp = top.enter_context(tc.tile_pool(name="wv2", bufs=1))
        wv_sb = wvp.tile([128, NB * 512], f16, name="wv_sb")
        h3p = top.enter_context(tc.tile_pool(name="hs3p", bufs=1))
        with tc.tile_pool(name="w", bufs=1) as wpool, \
             tc.tile_pool(name="hs", bufs=2) as hpool, \
             tc.tile_pool(name="psAB", bufs=1, space="PSUM") as psAB:
            wq_sb = wpool.tile([128, NB * 512], f16, name="wq_sb")
            wk_sb = wpool.tile([128, NB * 512], f16, name="wk_sb")
            # staged input DMAs: wq and hs chunk 0 stream in matched,
            # interleaved d-range pieces so pass-0 matmul progress tracks
            # transfer progress instead of waiting on a monolithic load.
            hs_t0 = hpool.tile([128, NB * 512], f16, tag="hs")
            for lo, hi in ((0, 1), (1, 3), (3, 6), (6, 11), (11, 16)):
                dma_in(wq_sb[:, lo * 512:hi * 512], wq3[:, lo * 512:hi * 512])
                dma_in(hs_t0[:, lo * 512:hi * 512], hs4[:, lo * 512:hi * 512])
            dma_in(wk_sb[:], wk3[:])
            dma_in(wv_sb[:], wv3[:])
            dma_in(wo_sb[:], wo4[:])

            pass_idx = 0
            hs_t = hs_t0
            for ch in range(CH):
                if ch + 1 < CH:
                    if ch + 1 == CH - 1:
                        hs_nxt = h3p.tile([128, NB * 512], f16, name="hs3")
                        hs3_t = hs_nxt
                    else:
                        hs_nxt = hpool.tile([128, NB * 512], f16, tag="hs")
                    dma_in(hs_nxt[:], hs4[:, (ch + 1) * 8192:(ch + 2) * 8192])
                # q-pass, k-pass: out [128 c-feat, 512 sq], contract over d.
                # ch3 runs k first so the last pass's psum banks are freed
                # by q's vector copies rather than k's scalar copies.
                order = (("q", wq_sb, qT), ("k", wk_sb, kT))
                if ch == CH - 1:
                    order = (order[1], order[0])
                for which, w_t, dstT in order:
                    base = 4 * (pass_idx % 2)
                    ps = [psAB.tile([128, 512], f32, tag=f"T{base+i}", name=f"p{which}{i}")
                          for i in range(4)]
                    for d in range(NB):
                        for cb in range(4):
                            nc.tensor.matmul(
                                ps[cb][:],
                                lhsT=w_t[:, d * 512 + cb * 128:d * 512 + (cb + 1) * 128],
                                rhs=hs_t[:, d * 512:(d + 1) * 512],
                                start=(d == 0), stop=(d == NB - 1))
                    for cb in range(4):
                        dst = dstT[cb][:, ch * 512:(ch + 1) * 512]
                        if ch == CH - 1 and which == "q":
                            # final pass: split copies across engines so the
                            # AB->C psum-pool handoff barrier (which waits on
                            # the last copy) clears ~1.4us sooner.
                            if cb < 2:
                                nc.vector.tensor_copy(dst, ps[cb][:])
                            else:
                                nc.scalar.copy(dst, ps[cb][:])
                        elif which == "q":
                            nc.vector.tensor_copy(dst, ps[cb][:])
                        else:
                            nc.scalar.copy(dst, ps[cb][:])
                    pass_idx += 1
                # v-pass: out [128 s-pos, 512 c-feat], contract over d.
                # ch3's v-pass is deferred into phase C (head 0, half 0).
                if ch < CH - 1:
                    base = 4 * (pass_idx % 2)
                    pv = [psAB.tile([128, 512], f32, tag=f"T{base+j}", name=f"pv{j}")
                          for j in range(4)]
                    for d in range(NB):
                        for j in range(4):
                            nc.tensor.matmul(
                                pv[j][:],
                                lhsT=hs_t[:, d * 512 + j * 128:d * 512 + (j + 1) * 128],
                                rhs=wv_sb[:, d * 512:(d + 1) * 512],
                                start=(d == 0), stop=(d == NB - 1))
                    for j in range(4):
                        nc.vector.tensor_copy(v_sb[ch * 4 + j][:], pv[j][:])
                    pass_idx += 1
                hs_t = hs_nxt if ch + 1 < CH else None

        # ---- phase C: attention (+ interleaved D-partial for heads 0,1) --
        # Each head runs in two sq-halves of 1024 so the av accumulator is
        # a single 2-bank PSUM slot; the 2 banks this frees host a small
        # o-proj pipeline: during heads 2 and 3 (whose exp keeps the scalar
        # engine saturated and the PE ~25% idle), one (nb,ch) cell of the
        # wo-contraction over heads {0,1} runs per sk step and is flushed
        # fp16 to SBUF (o1). Phase D then only contracts heads {2,3} and
        # merges o1 during the psum read-out.
        o1_pool = top.enter_context(tc.tile_pool(name="o1", bufs=1))
        o1 = [o1_pool.tile([128, S], f16, tag=f"o1_{nb}", name=f"o1_{nb}")
              for nb in range(NB)]
        # aT gets its own tiles: with sq-halving, half 0's normalize writes
        # aT while qT[h] is still read by half 1's scores, so the v1 trick
        # of reusing qT[h]'s slot would deadlock the pool.
        aT_pool = top.enter_context(tc.tile_pool(name="aT", bufs=1))
        HF = S // 2
        with tc.tile_pool(name="esb", bufs=6) as epool, \
             tc.tile_pool(name="lwork", bufs=2) as lpool, \
             tc.tile_pool(name="ones", bufs=1) as onepool, \
             tc.tile_pool(name="psS", bufs=2, space="PSUM") as psS, \
             tc.tile_pool(name="psO", bufs=1, space="PSUM") as psO, \
             tc.tile_pool(name="psD1", bufs=2, space="PSUM") as psD1:
            ones_f = onepool.tile([128, 128], f32, name="ones_f")
            nc.vector.memset(ones_f[:], 1.0)
            ones = onepool.tile([128, 128], f16, name="ones")
            nc.vector.tensor_copy(ones[:], ones_f[:])
            for h in range(4):
                aT = aT_pool.tile([128, S], f16, tag=f"aT{h}", name=f"aT{h}")
                deferred_flush = []
                for half in range(2):
                    qo = half * HF
                    # head 0, half 0 carries the deferred ch3 v-pass: 4
                    # v-matmuls per sk step fill the PE while exp runs; the
                    # av matmuls for sk 12-15 (whose v rows are produced
                    # here) are deferred past v completion so an in-order
                    # PE stall cannot deadlock against v production.
                    vfuse = (h == 0 and half == 0)
                    e_defer = {}
                    po = psO.tile([128, HF], f32, tag="po", name=f"po{h}_{half}")
                    acc = lpool.tile([128, HF], f16, tag="lw", name="acc")
                    for sk in range(NB):
                        ks = kT[h][:, sk * 128:(sk + 1) * 128]
                        ps = psS.tile([128, HF], f32, tag="ps", name="ps")
                        for c2 in range(2):
                            nc.tensor.matmul(
                                ps[:, c2 * 512:(c2 + 1) * 512],
                                lhsT=ks, rhs=qT[h][:, qo + c2 * 512:qo + (c2 + 1) * 512],
                                start=True, stop=True)
                        e_t = epool.tile([128, HF], f16, tag="esb")
                        nc.scalar.activation(e_t[:], ps[:],
                                             mybir.ActivationFunctionType.Exp,
                                             scale=float(SCALE))
                        vs = v_sb[sk][:, h * 128:(h + 1) * 128]
                        if vfuse and sk >= 12:
                            e_defer[sk] = e_t
                        else:
                            for c2 in range(2):
                                nc.tensor.matmul(
                                    po[:, c2 * 512:(c2 + 1) * 512], lhsT=vs,
                                    rhs=e_t[:, c2 * 512:(c2 + 1) * 512],
                                    start=(sk == 0),
                                    stop=(not vfuse and sk == NB - 1))
                        if vfuse:
                            for t in range(4):
                                vi = sk * 4 + t
                                j, dd = divmod(vi, NB)
                                if dd == 0:
                                    pv_c = psD1.tile([128, 512], f32, tag="pp",
                                                     name=f"pvc{j}")
                                nc.tensor.matmul(
                                    pv_c[:],
                                    lhsT=hs3_t[:, dd * 512 + j * 128:dd * 512 + (j + 1) * 128],
                                    rhs=wv_sb[:, dd * 512:(dd + 1) * 512],
                                    start=(dd == 0), stop=(dd == NB - 1))
                                if dd == NB - 1:
                                    nc.vector.tensor_copy(v_sb[12 + j][:], pv_c[:])
                        if h >= 2:
                            # one o-proj cell (heads 0,1) per step in the
                            # spare psum banks; flushes fp16 to o1, mostly
                            # on vector but every 4th on scalar (which has
                            # ~180ns/step of slack under the PE-bound rate).
                            ci = (h - 2) * 32 + half * 16 + sk
                            nb_i, ch_i = divmod(ci, 4)
                            pp = psD1.tile([128, 512], f32, tag="pp", name="pp")
                            for cb in (0, 1):
                                nc.tensor.matmul(
                                    pp[:],
                                    lhsT=wo_sb[:, cb * 2048 + nb_i * 128:cb * 2048 + (nb_i + 1) * 128],
                                    rhs=attnT[cb][:, ch_i * 512:(ch_i + 1) * 512],
                                    start=(cb == 0), stop=(cb == 1))
                            dst = o1[nb_i][:, ch_i * 512:(ch_i + 1) * 512]
                            if h == 3 and half == 1 and sk >= NB - 2:
                                # keep the last flushes out of vector's
                                # queue ahead of the reciprocal chain that
                                # gates phase D's psum banks.
                                deferred_flush.append((dst, pp))
                            else:
                                nc.vector.tensor_copy(dst, pp[:])
                        # acc skips sk15: its contribution enters the l-sum
                        # directly via a second accumulating ones-matmul, so
                        # the reciprocal chain starts one period earlier.
                        if sk == 0:
                            nc.vector.tensor_copy(acc[:], e_t[:])
                        elif sk < NB - 1:
                            nc.vector.tensor_add(acc[:], acc[:], e_t[:])
                        else:
                            e_last = e_t
                    # l-sums: for heads 0,1 they live in the (otherwise
                    # idle) psD1 banks so the scores ping-pong never waits
                    # on the reciprocal chain; for heads 2,3 psD1 hosts the
                    # o-proj cells (which must not wait on recips — a
                    # stalled cell matmul blocks the in-order PE), so the
                    # l-sums use a scores slot there instead.
                    # aT = po * (1/l) is fused into the psum read-out.
                    if h < 2:
                        pl = [psD1.tile([128, 512], f32, tag="pp", name=f"pl{i}")
                              for i in range(2)]
                        pls = [pl[0][:], pl[1][:]]
                    else:
                        plw = psS.tile([128, HF], f32, tag="ps", name="pl")
                        pls = [plw[:, 0:512], plw[:, 512:1024]]
                    for c2 in range(2):
                        nc.tensor.matmul(pls[c2], lhsT=ones[:],
                                         rhs=acc[:, c2 * 512:(c2 + 1) * 512],
                                         start=True, stop=False)
                        nc.tensor.matmul(pls[c2], lhsT=ones[:],
                                         rhs=e_last[:, c2 * 512:(c2 + 1) * 512],
                                         start=False, stop=True)
                    # catch-up av for the v rows produced in this half
                    for sk in sorted(e_defer):
                        vs = v_sb[sk][:, h * 128:(h + 1) * 128]
                        for c2 in range(2):
                            nc.tensor.matmul(
                                po[:, c2 * 512:(c2 + 1) * 512], lhsT=vs,
                                rhs=e_defer[sk][:, c2 * 512:(c2 + 1) * 512],
                                start=False, stop=(sk == NB - 1))
                    rcp = lpool.tile([128, HF], f32, tag="lwf", name="rcp")
                    for c2 in range(2):
                        nc.vector.reciprocal_approx_fast(
                            rcp[:, c2 * 512:(c2 + 1) * 512], pls[c2])
                    nc.vector.tensor_mul(aT[:, qo:qo + HF], po[:], rcp[:])
                    for dst, pp_d in deferred_flush:
                        nc.vector.tensor_copy(dst, pp_d[:])
                    deferred_flush = []
                attnT.append(aT)

        # ---- phase D: o-proj for heads {2,3} + merge with o1 (fp16 out) --
        with tc.tile_pool(name="osb", bufs=3) as opool, \
             tc.tile_pool(name="tmp", bufs=2) as tpool, \
             tc.tile_pool(name="psD", bufs=2, space="PSUM") as psD:
            for nb in range(NB):
                pp = [psD.tile([128, 1024], f32, tag=f"pp{i}", name=f"pp{i}") for i in range(2)]
                for cb in (2, 3):
                    for ch in range(CH):
                        nc.tensor.matmul(
                            pp[ch // 2][:, (ch % 2) * 512:(ch % 2) * 512 + 512],
                            lhsT=wo_sb[:, cb * 2048 + nb * 128:cb * 2048 + (nb + 1) * 128],
                            rhs=attnT[cb][:, ch * 512:(ch + 1) * 512],
                            start=(cb == 2), stop=(cb == 3))
                o_t = opool.tile([128, S], f16, tag="osb")
                # merge heads{2,3} psum with o1 (heads{0,1}) -> fp16 out.
                # Balance the slow 1-elem/cycle psum reads across scalar and
                # vector: every 3rd block goes scalar-copy x2 + one wide
                # fp16 add; the rest split one psum-TT on vector and one
                # scalar copy + fp16 add. Both engines land ~25us < PE 27.7.
                if nb % 3 == 0:
                    tmp2 = tpool.tile([128, S], f16, tag="tmp2")
                    nc.scalar.copy(tmp2[:, 0:1024], pp[0][:])
                    nc.scalar.copy(tmp2[:, 1024:2048], pp[1][:])
                    nc.vector.tensor_add(o_t[:], tmp2[:], o1[nb][:])
                else:
                    nc.vector.tensor_add(o_t[:, 0:1024], pp[0][:], o1[nb][:, 0:1024])
                    tmp = tpool.tile([128, 1024], f16, tag="tmp")
                    nc.scalar.copy(tmp[:], pp[1][:])
                    nc.vector.tensor_add(o_t[:, 1024:2048], tmp[:], o1[nb][:, 1024:2048])
                if nb >= NB - 2:
                    # last blocks: per-half DMAs so the tail isn't one
                    # full-row transfer behind the final merges.
                    for i in range(2):
                        dma_out(outT[nb * 128:(nb + 1) * 128, i * 1024:(i + 1) * 1024],
                                o_t[:, i * 1024:(i + 1) * 1024])
                else:
                    dma_out(outT[nb * 128:(nb + 1) * 128, :], o_t[:])

    nc.compile()
    return nc


def _get_program():
    if "nc" not in _BUILT:
        _BUILT["nc"] = _build_program()
    return _BUILT["nc"]


def _make_in_maps(hs, Wq, Wk, Wv, Wo):
    """Host-side pack of full fp32 inputs into 8 per-core fp16 tile layouts."""
    in_maps = []
    hs16 = [None] * B
    for b in range(B):
        hs16[b] = np.ascontiguousarray(
            hs[b].T.reshape(NB, 128, CH, 512).transpose(1, 2, 0, 3).reshape(128, CH * NB * 512)
        ).astype(np.float16)
    for c in range(8):
        b, g = divmod(c, 4)
        sl = slice(g * C, (g + 1) * C)
        wq = Wq[sl, :].T.reshape(NB, 128, C).transpose(1, 0, 2).reshape(128, NB * C)
        wk = Wk[sl, :].T.reshape(NB, 128, C).transpose(1, 0, 2).reshape(128, NB * C)
        wv = Wv[sl, :].T.reshape(NB, 128, C).transpose(1, 0, 2).reshape(128, NB * C)
        wo = Wo[:, sl].T.reshape(4, 128, S).transpose(1, 0, 2).reshape(128, 4 * S)
        in_maps.append({
            "hs4": hs16[b],
            "wq3": np.ascontiguousarray(wq).astype(np.float16),
            "wk3": np.ascontiguousarray(wk).astype(np.float16),
            "wv3": np.ascontiguousarray(wv).astype(np.float16),
            "wo4": np.ascontiguousarray(wo).astype(np.float16),
        })
    return in_maps


def _reference_fallback(hidden_states, attention_mask, Wq, bq, Wk, bk, Wv, bv, Wo, bo):
    q = hidden_states @ Wq.T + bq
    k = hidden_states @ Wk.T + bk
    v = hidden_states @ Wv.T + bv
    q = q.reshape(B, S, H, HD).transpose(0, 2, 1, 3)
    k = k.reshape(B, S, H, HD).transpose(0, 2, 1, 3)
    v = v.reshape(B, S, H, HD).transpose(0, 2, 1, 3)
    scores = np.einsum("bhqd,bhkd->bhqk", q, k) / np.sqrt(np.float32(HD))
    scores = scores + attention_mask
    scores -= scores.max(axis=-1, keepdims=True)
    e = np.exp(scores)
    attn = e / e.sum(axis=-1, keepdims=True)
    out = np.einsum("bhqk,bhkd->bhqd", attn, v)
    out = out.transpose(0, 2, 1, 3).reshape(B, S, D)
    return (out @ Wo.T + bo).astype(np.float32)


def kernel(hidden_states, attention_mask, Wq, bq, Wk, bk, Wv, bv, Wo, bo):
    from concourse import bass_utils

    hs = np.ascontiguousarray(np.asarray(hidden_states, dtype=np.float32))
    mask = np.asarray(attention_mask, dtype=np.float32)
    Wq = np.asarray(Wq, dtype=np.float32)
    Wk = np.asarray(Wk, dtype=np.float32)
    Wv = np.asarray(Wv, dtype=np.float32)
    Wo = np.asarray(Wo, dtype=np.float32)
    bq = np.asarray(bq, dtype=np.float32)
    bk = np.asarray(bk, dtype=np.float32)
    bv = np.asarray(bv, dtype=np.float32)
    bo = np.asarray(bo, dtype=np.float32)

    # Device program hardcodes zero mask / zero qkv biases (true for this
    # problem's setup_inputs); fall back to exact math if that ever changes.
    if mask.any() or bq.any() or bk.any() or bv.any():
        return _reference_fallback(hs, mask, Wq, bq, Wk, bk, Wv, bv, Wo, bo)

    nc = _get_program()
    in_maps = _make_in_maps(hs, Wq, Wk, Wv, Wo)
    res = bass_utils.run_bass_kernel_spmd(nc, in_maps, core_ids=list(range(8)))

    out = np.empty((B, S, D), dtype=np.float32)
    for b in range(B):
        accT = res.results[b * 4 + 0]["outT"].astype(np.float32)
        for g in range(1, 4):
            accT = accT + res.results[b * 4 + g]["outT"].astype(np.float32)
        out[b] = accT.T + bo
    return out


# revision 63
# speedup vs baseline: 1.0090x; 1.0028x over previous
"""Multi-head attention (B=2, S=2048, D=2048, H=16) on 8 Trainium2 cores.

Sharding: core c handles batch b=c//4 and head-group g=c%4 (4 heads, 512
features). Everything on-device runs in a transposed layout so the PE
contraction dim is always the partition dim:

  qT,kT [C=512, S]   = W[g].T-slices applied to hsT
  v     [S, C=512]
  scoresT[sk, sq]    = kT_h.T @ qT_h        (per head, per sq-chunk)
  p = exp(scoresT/sqrt(128))                (no max-subtract: scores are O(5))
  outT_h[c, sq]      = v_h.T @ p            (accumulated over sk blocks)
  l_h[sq] = sum_sk p -> r=1/l, attnT_h *= r
  partialT[n, sq]    = wo_g.T @ attnT       (per-core partial of o-proj)

Host sums the 4 per-batch partials (fp16 on wire), transposes back, adds bo.
All matmuls run in fp16 (4x fp32 rate).

Phases A (q/k-proj) and B (v-proj) are merged: per 512-column chunk of
hsT we run a q-pass, k-pass and v-pass against the same SBUF-resident hs
tile (PSUM banks rotate between passes), so hs is loaded once. Inputs are
DMA'd in a handful of large descriptors on the sync queue (host pre-packs
the tile layout); outputs go out in fp16 per 128-row block on gpsimd.
"""
import sys

if "/opt/trn_rl_repo" not in sys.path:
    sys.path.insert(0, "/opt/trn_rl_repo")

import numpy as np

B, S, D, H = 2, 2048, 2048, 16
HD = 128          # head dim
C = 512           # features per core (4 heads)
NB = S // 128     # 16 partition blocks
CH = S // 512     # 4 free-dim chunks
SCALE = 1.0 / np.sqrt(HD)

_BUILT = {}


def _build_program():
    import concourse.bass as bass
    import concourse.tile as tile
    from concourse import bacc, mybir, bass_isa
    from contextlib import ExitStack

    f32 = mybir.dt.float32
    f16 = mybir.dt.float16

    nc = bacc.Bacc("TRN2", target_bir_lowering=False, debug=False, num_devices=1)
    # Host pre-packs every input into the exact SBUF tile layout so each
    # DMA below is a plain 2D contiguous transfer.
    #   hs4[p, ch*8192 + d*512 + c] = hsT[d*128+p, ch*512+c]
    #   wq3/wk3/wv3[p, d*512 + c]   = W.T[d*128+p, c]          (per-core 512 cols)
    #   wo4[p, cb*2048 + n]         = Wo_g.T[cb*128+p, n]
    hs4 = nc.dram_tensor("hs4", (128, CH * NB * 512), f16, kind="ExternalInput").ap()
    wq3 = nc.dram_tensor("wq3", (128, NB * 512), f16, kind="ExternalInput").ap()
    wk3 = nc.dram_tensor("wk3", (128, NB * 512), f16, kind="ExternalInput").ap()
    wv3 = nc.dram_tensor("wv3", (128, NB * 512), f16, kind="ExternalInput").ap()
    wo4 = nc.dram_tensor("wo4", (128, 4 * S), f16, kind="ExternalInput").ap()
    outT = nc.dram_tensor("outT", (D, S), f16, kind="ExternalOutput").ap()

    with tile.TileContext(nc) as tc, ExitStack() as top:
        dma_in = nc.sync.dma_start
        dma_out = nc.gpsimd.dma_start

        # ---- persistent activations -------------------------------------
        # attnT[h] reuses qT[h]'s slot (same tag): qT[h] is dead after head
        # h's last scores matmul, right when attnT[h] starts filling.
        qk_pool = top.enter_context(tc.tile_pool(name="qk", bufs=1))
        qT = [qk_pool.tile([128, S], f16, tag=f"qT{cb}", name=f"qT{cb}") for cb in range(4)]
        kT = [qk_pool.tile([128, S], f16, tag=f"kT{cb}", name=f"kT{cb}") for cb in range(4)]
        v_pool = top.enter_context(tc.tile_pool(name="v", bufs=1))
        v_sb = [v_pool.tile([128, C], f16, tag=f"v{sb}", name=f"v{sb}") for sb in range(NB)]
        wop = top.enter_context(tc.tile_pool(name="wo", bufs=1))
        wo_sb = wop.tile([128, 4 * S], f16, name="wo_sb")
        attnT = []

        # ---- phase AB: qT, kT, v ----------------------------------------
        # wv and the last hs chunk outlive phase AB: the ch3 v-pass is
        # interleaved into head 0's first half (phase C), where the scalar
        # exp otherwise leaves the PE ~25% idle.
        wvp = top.enter_context(tc.tile_pool(name="wv2", bufs=1))
        wv_sb = wvp.tile([128, NB * 512], f16, name="wv_sb")
        h3p = top.enter_context(tc.tile_pool(name="hs3p", bufs=1))
        with tc.tile_pool(name="w", bufs=1) as wpool, \
             tc.tile_pool(name="hs", bufs=2) as hpool, \
             tc.tile_pool(name="psAB", bufs=1, space="PSUM") as psAB:
            wq_sb = wpool.tile([128, NB * 512], f16, name="wq_sb")
            wk_sb = wpool.tile([128, NB * 512], f16, name="wk_sb")
            # staged input DMAs: wq and hs chunk 0 stream in matched,
            # interleaved d-range pieces so pass-0 matmul progress tracks
            # transfer progress instead of waiting on a monolithic load.
            hs_t0 = hpool.tile([128, NB * 512], f16, tag="hs")
            for lo, hi in ((0, 1), (1, 3), (3, 6), (6, 11), (11, 16)):
                dma_in(wq_sb[:, lo * 512:hi * 512], wq3[:, lo * 512:hi * 512])
                dma_in(hs_t0[:, lo * 512:hi * 512], hs4[:, lo * 512:hi * 512])
            dma_in(wk_sb[:], wk3[:])
            dma_in(wv_sb[:], wv3[:])
            dma_in(wo_sb[:], wo4[:])

            pass_idx = 0
            hs_t = hs_t0
            for ch in range(CH):
                if ch + 1 < CH:
                    if ch + 1 == CH - 1:
                        hs_nxt = h3p.tile([128, NB * 512], f16, name="hs3")
                        hs3_t = hs_nxt
                    else:
                        hs_nxt = hpool.tile([128, NB * 512], f16, tag="hs")
                    dma_in(hs_nxt[:], hs4[:, (ch + 1) * 8192:(ch + 2) * 8192])
                # q-pass, k-pass: out [128 c-feat, 512 sq], contract over d.
                # ch3 runs k first so the last pass's psum banks are freed
                # by q's vector copies rather than k's scalar copies.
                order = (("q", wq_sb, qT), ("k", wk_sb, kT))
                if ch == CH - 1:
                    order = (order[1], order[0])
                for which, w_t, dstT in order:
                    base = 4 * (pass_idx % 2)
                    ps = [psAB.tile([128, 512], f32, tag=f"T{base+i}", name=f"p{which}{i}")
                          for i in range(4)]
                    for d in range(NB):
                        for cb in range(4):
                            nc.tensor.matmul(
                                ps[cb][:],
                                lhsT=w_t[:, d * 512 + cb * 128:d * 512 + (cb + 1) * 128],
                                rhs=hs_t[:, d * 512:(d + 1) * 512],
                                start=(d == 0), stop=(d == NB - 1))
                    for cb in range(4):
                        dst = dstT[cb][:, ch * 512:(ch + 1) * 512]
                        if ch == CH - 1 and which == "q":
                            # final pass: split copies across engines so the
                            # AB->C psum-pool handoff barrier (which waits on
                            # the last copy) clears ~1.4us sooner.
                            if cb < 2:
                                nc.vector.tensor_copy(dst, ps[cb][:])
                            else:
                                nc.scalar.copy(dst, ps[cb][:])
                        elif which == "q":
                            nc.vector.tensor_copy(dst, ps[cb][:])
                        else:
                            nc.scalar.copy(dst, ps[cb][:])
                    pass_idx += 1
                # v-pass: out [128 s-pos, 512 c-feat], contract over d.
                # ch3's v-pass is deferred into phase C (head 0, half 0).
                if ch < CH - 1:
                    base = 4 * (pass_idx % 2)
                    pv = [psAB.tile([128, 512], f32, tag=f"T{base+j}", name=f"pv{j}")
                          for j in range(4)]
                    for d in range(NB):
                        for j in range(4):
                            nc.tensor.matmul(
                                pv[j][:],
                                lhsT=hs_t[:, d * 512 + j * 128:d * 512 + (j + 1) * 128],
                                rhs=wv_sb[:, d * 512:(d + 1) * 512],
                                start=(d == 0), stop=(d == NB - 1))
                    for j in range(4):
                        nc.vector.tensor_copy(v_sb[ch * 4 + j][:], pv[j][:])
                    pass_idx += 1
                hs_t = hs_nxt if ch + 1 < CH else None

        # ---- phase C: attention (+ interleaved D-partial for heads 0,1) --
        # Each head runs in two sq-halves of 1024 so the av accumulator is
        # a single 2-bank PSUM slot; the 2 banks this frees host a small
        # o-proj pipeline: during heads 2 and 3 (whose exp keeps the scalar
        # engine saturated and the PE ~25% idle), one (nb,ch) cell of the
        # wo-contraction over heads {0,1} runs per sk step and is flushed
        # fp16 to SBUF (o1). Phase D then only contracts heads {2,3} and
        # merges o1 during the psum read-out.
        o1_pool = top.enter_context(tc.tile_pool(name="o1", bufs=1))
        o1 = [o1_pool.tile([128, S], f16, tag=f"o1_{nb}", name=f"o1_{nb}")
              for nb in range(NB)]
        # aT gets its own tiles: with sq-halving, half 0's normalize writes
        # aT while qT[h] is still read by half 1's scores, so the v1 trick
        # of reusing qT[h]'s slot would deadlock the pool.
        aT_pool = top.enter_context(tc.tile_pool(name="aT", bufs=1))
        HF = S // 2
        with tc.tile_pool(name="esb", bufs=6) as epool, \
             tc.tile_pool(name="lwork", bufs=2) as lpool, \
             tc.tile_pool(name="ones", bufs=1) as onepool, \
             tc.tile_pool(name="psS", bufs=2, space="PSUM") as psS, \
             tc.tile_pool(name="psO", bufs=1, space="PSUM") as psO, \
             tc.tile_pool(name="psD1", bufs=2, space="PSUM") as psD1:
            ones_f = onepool.tile([128, 128], f32, name="ones_f")
            nc.vector.memset(ones_f[:], 1.0)
            ones = onepool.tile([128, 128], f16, name="ones")
            nc.vector.tensor_copy(ones[:], ones_f[:])
            for h in range(4):
                aT = aT_pool.tile([128, S], f16, tag=f"aT{h}", name=f"aT{h}")
                deferred_flush = []
                for half in range(2):
                    qo = half * HF
                    # head 0, half 0 carries the deferred ch3 v-pass: 4
                    # v-matmuls per sk step fill the PE while exp runs; the
                    # av matmuls for sk 12-15 (whose v rows are produced
                    # here) are deferred past v completion so an in-order
                    # PE stall cannot deadlock against v production.
                    vfuse = (h == 0 and half == 0)
                    e_defer = {}
                    po = psO.tile([128, HF], f32, tag="po", name=f"po{h}_{half}")
                    acc = lpool.tile([128, HF], f16, tag="lw", name="acc")
                    for sk in range(NB):
                        ks = kT[h][:, sk * 128:(sk + 1) * 128]
                        ps = psS.tile([128, HF], f32, tag="ps", name="ps")
                        for c2 in range(2):
                            nc.tensor.matmul(
                                ps[:, c2 * 512:(c2 + 1) * 512],
                                lhsT=ks, rhs=qT[h][:, qo + c2 * 512:qo + (c2 + 1) * 512],
                                start=True, stop=True)
                        e_t = epool.tile([128, HF], f16, tag="esb")
                        nc.scalar.activation(e_t[:], ps[:],
                                             mybir.ActivationFunctionType.Exp,
                                             scale=float(SCALE))
                        vs = v_sb[sk][:, h * 128:(h + 1) * 128]
                        if vfuse and sk >= 12:
                            e_defer[sk] = e_t
                        else:
                            for c2 in range(2):
                                nc.tensor.matmul(
                                    po[:, c2 * 512:(c2 + 1) * 512], lhsT=vs,
                                    rhs=e_t[:, c2 * 512:(c2 + 1) * 512],
                                    start=(sk == 0),
                                    stop=(not vfuse and sk == NB - 1))
                        if vfuse:
                            for t in range(4):
                                vi = sk * 4 + t
                                j, dd = divmod(vi, NB)
                                if dd == 0:
                                    pv_c = psD1.tile([128, 512], f32, tag="pp",
                                                     name=f"pvc{j}")
                                nc.tensor.matmul(
                                    pv_c[:],
                                    lhsT=hs3_t[:, dd * 512 + j * 128:dd * 512 + (j + 1) * 128],
                                    rhs=wv_sb[:, dd * 512:(dd + 1) * 512],
                                    start=(dd == 0), stop=(dd == NB - 1))
                                if dd == NB - 1:
                                    nc.vector.tensor_copy(v_sb[12 + j][:], pv_c[:])
                        if h >= 2:
                            # one o-proj cell (heads 0,1) per step in the
                            # spare psum banks; flushes fp16 to o1, mostly
                            # on vector but every 4th on scalar (which has
                            # ~180ns/step of slack under the PE-bound rate).
                            ci = (h - 2) * 32 + half * 16 + sk
                            nb_i, ch_i = divmod(ci, 4)
                            pp = psD1.tile([128, 512], f32, tag="pp", name="pp")
                            for cb in (0, 1):
                                nc.tensor.matmul(
                                    pp[:],
                                    lhsT=wo_sb[:, cb * 2048 + nb_i * 128:cb * 2048 + (nb_i + 1) * 128],
                                    rhs=attnT[cb][:, ch_i * 512:(ch_i + 1) * 512],
                                    start=(cb == 0), stop=(cb == 1))
                            dst = o1[nb_i][:, ch_i * 512:(ch_i + 1) * 512]
                            if h == 3 and half == 1 and sk >= NB - 2:
                                # keep the last flushes out of vector's
                                # queue ahead of the reciprocal chain that
                                # gates phase D's psum banks.
                                deferred_flush.append((dst, pp))
                            else:
                                nc.vector.tensor_copy(dst, pp[:])
                        # acc skips sk15: its contribution enters the l-sum
                        # directly via a second accumulating ones-matmul, so
                        # the reciprocal chain starts one period earlier.
                        if sk == 0:
                            nc.vector.tensor_copy(acc[:], e_t[:])
                        elif sk < NB - 1:
                            nc.vector.tensor_add(acc[:], acc[:], e_t[:])
                        else:
                            e_last = e_t
                    # l-sums: for heads 0,1 they live in the (otherwise
                    # idle) psD1 banks so the scores ping-pong never waits
                    # on the reciprocal chain; for heads 2,3 psD1 hosts the
                    # o-proj cells (which must not wait on recips — a
                    # stalled cell matmul blocks the in-order PE), so the
                    # l-sums use a scores slot there instead.
                    # aT = po * (1/l) is fused into the psum read-out.
                    if h < 2:
                        pl = [psD1.tile([128, 512], f32, tag="pp", name=f"pl{i}")
                              for i in range(2)]
                        pls = [pl[0][:], pl[1][:]]
                    else:
                        plw = psS.tile([128, HF], f32, tag="ps", name="pl")
                        pls = [plw[:, 0:512], plw[:, 512:1024]]
                    for c2 in range(2):
                        nc.tensor.matmul(pls[c2], lhsT=ones[:],
                                         rhs=acc[:, c2 * 512:(c2 + 1) * 512],
                                         start=True, stop=False)
                        nc.tensor.matmul(pls[c2], lhsT=ones[:],
                                         rhs=e_last[:, c2 * 512:(c2 + 1) * 512],
                                         start=False, stop=True)
                    # catch-up av for the v rows produced in this half
                    for sk in sorted(e_defer):
                        vs = v_sb[sk][:, h * 128:(h + 1) * 128]
                        for c2 in range(2):
                            nc.tensor.matmul(
                                po[:, c2 * 512:(c2 + 1) * 512], lhsT=vs,
                                rhs=e_defer[sk][:, c2 * 512:(c2 + 1) * 512],
                                start=False, stop=(sk == NB - 1))
                    rcp = lpool.tile([128, HF], f32, tag="lwf", name="rcp")
                    for c2 in range(2):
                        nc.vector.reciprocal_approx_fast(
                            rcp[:, c2 * 512:(c2 + 1) * 512], pls[c2])
                    nc.vector.tensor_mul(aT[:, qo:qo + HF], po[:], rcp[:])
                    for dst, pp_d in deferred_flush:
                        nc.vector.tensor_copy(dst, pp_d[:])
                    deferred_flush = []
                attnT.append(aT)

        # ---- phase D: o-proj for heads {2,3} + merge with o1 (fp16 out) --
        with tc.tile_pool(name="osb", bufs=3) as opool, \
             tc.tile_pool(name="tmp", bufs=2) as tpool, \
             tc.tile_pool(name="psD", bufs=2, space="PSUM") as psD:
            for nb in range(NB):
                pp = [psD.tile([128, 1024], f32, tag=f"pp{i}", name=f"pp{i}") for i in range(2)]
                for cb in (2, 3):
                    for ch in range(CH):
                        nc.tensor.matmul(
                            pp[ch // 2][:, (ch % 2) * 512:(ch % 2) * 512 + 512],
                            lhsT=wo_sb[:, cb * 2048 + nb * 128:cb * 2048 + (nb + 1) * 128],
                            rhs=attnT[cb][:, ch * 512:(ch + 1) * 512],
                            start=(cb == 2), stop=(cb == 3))
                o_t = opool.tile([128, S], f16, tag="osb")
                # merge heads{2,3} psum with o1 (heads{0,1}) -> fp16 out.
                # Balance the slow 1-elem/cycle psum reads across scalar and
                # vector: every 3rd block goes scalar-copy x2 + one wide
                # fp16 add; the rest split one psum-TT on vector and one
                # scalar copy + fp16 add. Both engines land ~25us < PE 27.7.
                if nb % 2 == 0 and nb < 14:
                    tmp2 = tpool.tile([128, S], f16, tag="tmp2")
                    nc.scalar.copy(tmp2[:, 0:1024], pp[0][:])
                    nc.scalar.copy(tmp2[:, 1024:2048], pp[1][:])
                    nc.vector.tensor_add(o_t[:], tmp2[:], o1[nb][:])
                else:
                    nc.vector.tensor_add(o_t[:, 0:1024], pp[0][:], o1[nb][:, 0:1024])
                    tmp = tpool.tile([128, 1024], f16, tag="tmp")
                    nc.scalar.copy(tmp[:], pp[1][:])
                    nc.vector.tensor_add(o_t[:, 1024:2048], tmp[:], o1[nb][:, 1024:2048])
                if nb >= NB - 2:
                    # last blocks: per-half DMAs so the tail isn't one
                    # full-row transfer behind the final merges.
                    for i in range(2):
                        dma_out(outT[nb * 128:(nb + 1) * 128, i * 1024:(i + 1) * 1024],
                                o_t[:, i * 1024:(i + 1) * 1024])
                else:
                    dma_out(outT[nb * 128:(nb + 1) * 128, :], o_t[:])

    nc.compile()
    return nc


def _get_program():
    if "nc" not in _BUILT:
        _BUILT["nc"] = _build_program()
    return _BUILT["nc"]


def _make_in_maps(hs, Wq, Wk, Wv, Wo):
    """Host-side pack of full fp32 inputs into 8 per-core fp16 tile layouts."""
    in_maps = []
    hs16 = [None] * B
    for b in range(B):
        hs16[b] = np.ascontiguousarray(
            hs[b].T.reshape(NB, 128, CH, 512).transpose(1, 2, 0, 3).reshape(128, CH * NB * 512)
        ).astype(np.float16)
    for c in range(8):
        b, g = divmod(c, 4)
        sl = slice(g * C, (g + 1) * C)
        wq = Wq[sl, :].T.reshape(NB, 128, C).transpose(1, 0, 2).reshape(128, NB * C)
        wk = Wk[sl, :].T.reshape(NB, 128, C).transpose(1, 0, 2).reshape(128, NB * C)
        wv = Wv[sl, :].T.reshape(NB, 128, C).transpose(1, 0, 2).reshape(128, NB * C)
        wo = Wo[:, sl].T.reshape(4, 128, S).transpose(1, 0, 2).reshape(128, 4 * S)
        in_maps.append({
            "hs4": hs16[b],
            "wq3": np.ascontiguousarray(wq).astype(np.float16),
            "wk3": np.ascontiguousarray(wk).astype(np.float16),
            "wv3": np.ascontiguousarray(wv).astype(np.float16),
            "wo4": np.ascontiguousarray(wo).astype(np.float16),
        })
    return in_maps


def _reference_fallback(hidden_states, attention_mask, Wq, bq, Wk, bk, Wv, bv, Wo, bo):
    q = hidden_states @ Wq.T + bq
    k = hidden_states @ Wk.T + bk
    v = hidden_states @ Wv.T + bv
    q = q.reshape(B, S, H, HD).transpose(0, 2, 1, 3)
    k = k.reshape(B, S, H, HD).transpose(0, 2, 1, 3)
    v = v.reshape(B, S, H, HD).transpose(0, 2, 1, 3)
    scores = np.einsum("bhqd,bhkd->bhqk", q, k) / np.sqrt(np.float32(HD))
    scores = scores + attention_mask
    scores -= scores.max(axis=-1, keepdims=True)
    e = np.exp(scores)
    attn = e / e.sum(axis=-1, keepdims=True)
    out = np.einsum("bhqk,bhkd->bhqd", attn, v)
    out = out.transpose(0, 2, 1, 3).reshape(B, S, D)
    return (out @ Wo.T + bo).astype(np.float32)


def kernel(hidden_states, attention_mask, Wq, bq, Wk, bk, Wv, bv, Wo, bo):
    from concourse import bass_utils

    hs = np.ascontiguousarray(np.asarray(hidden_states, dtype=np.float32))
    mask = np.asarray(attention_mask, dtype=np.float32)
    Wq = np.asarray(Wq, dtype=np.float32)
    Wk = np.asarray(Wk, dtype=np.float32)
    Wv = np.asarray(Wv, dtype=np.float32)
    Wo = np.asarray(Wo, dtype=np.float32)
    bq = np.asarray(bq, dtype=np.float32)
    bk = np.asarray(bk, dtype=np.float32)
    bv = np.asarray(bv, dtype=np.float32)
    bo = np.asarray(bo, dtype=np.float32)

    # Device program hardcodes zero mask / zero qkv biases (true for this
    # problem's setup_inputs); fall back to exact math if that ever changes.
    if mask.any() or bq.any() or bk.any() or bv.any():
        return _reference_fallback(hs, mask, Wq, bq, Wk, bk, Wv, bv, Wo, bo)

    nc = _get_program()
    in_maps = _make_in_maps(hs, Wq, Wk, Wv, Wo)
    res = bass_utils.run_bass_kernel_spmd(nc, in_maps, core_ids=list(range(8)))

    out = np.empty((B, S, D), dtype=np.float32)
    for b in range(B):
        accT = res.results[b * 4 + 0]["outT"].astype(np.float32)
        for g in range(1, 4):
            accT = accT + res.results[b * 4 + g]["outT"].astype(np.float32)
        out[b] = accT.T + bo
    return out
